# revision 65
# baseline (speedup 1.0000x reference)
"""AxonLIFNode forward on 8 Trainium2 NeuronCores.

Reference recurrence (per element, sequential over T):
    mem   = mem + (x_t + V_RESET - mem) / TAU        # V_RESET=0, TAU=2
    spike = (mem - V_TH > 0)                         # V_TH=1, {0.0, 1.0}
    mem   = (1 - spike) * mem + V_RESET * spike      # reset to 0 on spike
    out_i = out_i * sigmoid(w) + spike               # axon current (w=0 -> 0.5)
    outputs: (spike, out_i), both [B, T, N] f32

Strategy: data-parallel over the batch axis (B=64 -> 8 per core). Per core the
32768 independent series are laid out as 128 partitions x 256 free elements;
the T=64 loop runs as a serial chain of one fused custom-DVE instruction per
timestep computing the pre-reset membrane m1_t from (x_t, m1_{t-1}):

    prev = m1 * (m1 <= 1)            # reset of the *previous* pre-reset mem
    m1'  = prev + (x - prev) * 0.5

which is bit-exact vs. the reference ordering (each ALU stage is one IEEE f32
rounding; *0.5 == /2 exactly). A second fused DVE op produces the axon current
oi_t = oi_{t-1}*inv_tau + (m1_t > 1) directly from m1, so the DVE critical
path is exactly two 1x-mode instructions per timestep (the ISA floor). Spikes
are produced off the critical path on the Scalar(ACT) engine with a saturated
sigmoid (exact {0,1}, see _build) and stored as fp8. X streams in on the SP
HWDGE ring in tiered batches; spikes stream out per-G-timestep group on the
ACT ring while the heavier oi stream is staged across two groups (1 MB
stores, late ones on the SP ring once inputs finish).
"""

import numpy as np

import concourse.bacc as bacc
import concourse.mybir as mybir
import concourse.dve_ops as dve_ops
from concourse.dve_ops import DveOp
from concourse.dve_spec import Spec, Src0, Src1, C0, C1, lower
from concourse.dve_uop import DveOpSpec
from concourse.tile import TileContext
from concourse.bass_utils import run_bass_kernel_spmd

# Problem shape (hardcoded per harness contract).
B, T, N = 64, 64, 4096
CORES = 8
BS = B // CORES          # batches per core
P = 128                  # SBUF partitions
J = 16                   # n-chunks per batch: BS * J == P
F = N // J               # free elements per partition per timestep (256)
G = 4                    # timesteps per output staging group
GIN = 16                 # timesteps per input DMA batch
SPK_SCALE = 1.0e9        # sigmoid saturation trick scale (see _build)

def _register_op(name: str, spec: Spec) -> DveOp:
    """Register a custom DVE op in the global registry with a computed sha."""
    for op in dve_ops.OPS:
        if op.name == name:
            return op
    row = dve_ops._CUSTOM_DVE_ROW_BASE + len(dve_ops.OPS)
    assert row < 0x20, "custom-DVE opcode rows exhausted"
    shas = {}
    for ver in ("v3", "v4"):
        uops = lower(spec, ver=ver)
        shas[ver] = DveOpSpec(name=name, opcode=row, uops=uops, rd1_en=True).sha(ver)
    op = DveOp(name, spec, subdim=False, uops_sha=shas)
    dve_ops._SUB_OPCODE_FOR_NAME[name] = row
    dve_ops.OPS.append(op)
    dve_ops.CUSTOM_DVE_SPECS[name] = spec
    return op


def _lif_ops() -> tuple[DveOp, DveOp]:
    """LIF_M1: m1_t from (x_t, m1_{t-1}); LIF_OI: oi_t from (oi_{t-1}, m1_t).

    LIF_M1: out = prev + (Src0 - prev) * C0, prev = Src1 * (Src1 <= C1)
    LIF_OI: out = Src0 * C0 + (Src1 > C1)
    Each ALU stage is one IEEE f32 rounding; bit-exact vs the reference.
    """
    keep = Src1 <= C1
    prev = Src1 * keep
    m1 = _register_op(
        "LIF_M1_ANT",
        Spec(
            body=prev + (Src0 - prev) * C0,
            reference=lambda in0, in1, s0, s1, imm2: (
                (p := (in1 * (in1 <= s1)).astype(np.float32))
                + (in0 - p) * np.float32(s0)
            ).astype(np.float32),
        ),
    )
    oi = _register_op(
        "LIF_OI_ANT",
        Spec(
            body=Src0 * C0 + (Src1 > C1),
            reference=lambda in0, in1, s0, s1, imm2: (
                in0 * np.float32(s0) + (in1 > s1)
            ).astype(np.float32),
        ),
    )
    return m1, oi


_nc_cache: dict = {}


def _build(inv_tau: float):
    """Trace + compile the per-core Bass program (SPMD: same NEFF, 8 cores)."""
    key = float(inv_tau)
    if key in _nc_cache:
        return _nc_cache[key]

    lif_m1, lif_oi = _lif_ops()
    f32 = mybir.dt.float32
    fp8 = mybir.dt.float8e4

    nc = bacc.Bacc(
        "TRN2",
        target_bir_lowering=False,
        debug=False,
        enable_asserts=False,
        num_devices=CORES,
    )
    # Host pre-transposes each core's shard to [(b j) = 128, T, F] contiguous,
    # so every DMA is a 3-dim AP with an 8 KB contiguous run per partition.
    x_r = nc.dram_tensor("x", [P, T, F], f32, kind="ExternalInput").ap()
    # Spikes are exactly {0.0, 1.0}: store as fp8-e4m3 (lossless) to cut the
    # HBM write traffic 4x; the host upcasts to f32.
    spk_r = nc.dram_tensor("spk", [P, T, F], fp8, kind="ExternalOutput").ap()
    oi_r = nc.dram_tensor("oi", [P, T, F], f32, kind="ExternalOutput").ap()

    with TileContext(nc) as tc:
        with (
            tc.tile_pool(name="const", bufs=1) as cpool,
            tc.tile_pool(name="xin", bufs=1) as xpool,
            tc.tile_pool(name="sout", bufs=6) as spool,
            tc.tile_pool(name="oout", bufs=8) as opool,
            tc.tile_pool(name="mstate", bufs=10) as mpool,
        ):
            m_init = cpool.tile([P, F], f32)
            nc.vector.memset(m_init[:], 0.0)
            oi_init = cpool.tile([P, F], f32)
            nc.vector.memset(oi_init[:], 0.0)
            # Spike via one ACT op: sigmoid(S*m1 - (S + 64)) with S = 1e9.
            # fl(S*m1) quantizes to a 64-ulp grid around S, so the argument is
            # always <= -64 (no spike, incl. m1 == V_TH exactly -> -64) or
            # >= +64 (spike); sigmoid saturates to 0.0 / 1.0 there.
            spk_bias = cpool.tile([P, 1], f32)
            nc.vector.memset(spk_bias[:], -(SPK_SCALE + 64.0))
            m_prev = m_init[:]
            oi_prev = oi_init[:]

            # The whole per-core X fits in SBUF (64 KB/partition). Stage it in
            # tiered batches — small first chunks so compute starts early —
            # and issue every input DMA up front on the dedicated SP ring.
            in_batches = [1, 3, 4, 8] + [GIN] * ((T - 16) // GIN)
            assert sum(in_batches) == T
            x_tiles = []  # (tile, t_start, t_len)
            t_cursor = 0
            for blen in in_batches:
                xt = xpool.tile([P, blen, F], f32, name=f"x_{t_cursor}", bufs=1)
                x_tiles.append((xt, t_cursor, blen))
                t_cursor += blen

            issued = set()

            def issue_x_until(t_max):
                # Pace input DMAs so they don't flood the SDMA engines and
                # starve the output stream. The first small batches alternate
                # between the two HWDGE rings (the ACT ring is idle until the
                # first outputs), so their transfers run in parallel and the
                # DVE chain never waits at an early batch boundary.
                for bi, (xt, ts, tl) in enumerate(x_tiles):
                    if ts < t_max and bi not in issued:
                        issued.add(bi)
                        nc.sync.dma_start(out=xt[:], in_=x_r[:, ts : ts + tl, :])

            def x_slice(t):
                for xt, ts, tl in x_tiles:
                    if ts <= t < ts + tl:
                        return xt[:, t - ts, :]
                raise AssertionError(t)

            issue_x_until(16)
            OG = 2 * G  # oi staging spans two groups: 1 MB stores
            o_t = None
            for g in range(T // G):
                issue_x_until((g + 3) * G)
                s_t = spool.tile([P, G, F], fp8)
                if g % 2 == 0:
                    o_t = opool.tile([P, OG, F], f32)
                koff = (g % 2) * G
                m_t = mpool.tile([P, G, F], f32)
                for k in range(G):
                    # DVE: m1_t = prev + (x_t - prev)/TAU, prev = reset(m1_{t-1})
                    nc.vector._custom_dve(
                        lif_m1,
                        out=m_t[:, k, :],
                        in0=x_slice(g * G + k),
                        in1=m_prev,
                        s0=0.5,      # 1/TAU
                        s1=1.0,      # V_TH
                    )
                    # DVE: oi_t = oi_{t-1} * inv_tau + (m1_t > V_TH)
                    nc.vector._custom_dve(
                        lif_oi,
                        out=o_t[:, koff + k, :],
                        in0=oi_prev,
                        in1=m_t[:, k, :],
                        s0=inv_tau,
                        s1=1.0,
                    )
                    m_prev = m_t[:, k, :]
                    oi_prev = o_t[:, koff + k, :]
                # ACT: spike = sigmoid(S*m1 - (S+64)) in {0, 1}. One batched op
                # per group; the final group goes half-at-a-time so the last
                # spike store isn't serialized behind a full-group ACT op.
                if g == T // G - 1:
                    h = G // 2
                    nc.scalar.activation(
                        out=s_t[:, :h, :],
                        in_=m_t[:, :h, :],
                        func=mybir.ActivationFunctionType.Sigmoid,
                        bias=spk_bias[:],
                        scale=SPK_SCALE,
                    )
                    nc.scalar.activation(
                        out=s_t[:, h:, :],
                        in_=m_t[:, h:, :],
                        func=mybir.ActivationFunctionType.Sigmoid,
                        bias=spk_bias[:],
                        scale=SPK_SCALE,
                    )
                else:
                    nc.scalar.activation(
                        out=s_t[:],
                        in_=m_t[:],
                        func=mybir.ActivationFunctionType.Sigmoid,
                        bias=spk_bias[:],
                        scale=SPK_SCALE,
                    )
                # Early groups: outputs on the ACT ring while inputs own the SP
                # ring; once inputs are done the SP ring takes the oi stream.
                if g == T // G - 1:
                    h = G // 2
                    nc.scalar.dma_start(
                        out=spk_r[:, g * G : g * G + h, :], in_=s_t[:, :h, :]
                    )
                    nc.scalar.dma_start(
                        out=spk_r[:, g * G + h : (g + 1) * G, :], in_=s_t[:, h:, :]
                    )
                else:
                    nc.scalar.dma_start(
                        out=spk_r[:, g * G : (g + 1) * G, :], in_=s_t[:]
                    )
                if g == T // G - 1:
                    # Final stores split across both rings so the tail drains
                    # in parallel.
                    nc.sync.dma_start(
                        out=oi_r[:, g * G - G : g * G, :], in_=o_t[:, :G, :]
                    )
                    nc.scalar.dma_start(
                        out=oi_r[:, g * G : (g + 1) * G, :], in_=o_t[:, G:, :]
                    )
                elif g % 2 == 1:
                    oi_eng = nc.sync if g >= (T // G) // 2 else nc.scalar
                    oi_eng.dma_start(
                        out=oi_r[:, (g - 1) * G : (g + 1) * G, :], in_=o_t[:]
                    )

    nc.compile()
    _nc_cache[key] = nc
    return nc


def _shard(X: np.ndarray) -> list[np.ndarray]:
    """[B, T, N] -> per-core [(b j) = 128, T, F] contiguous."""
    Xt = np.ascontiguousarray(
        X.reshape(B, T, J, F).transpose(0, 2, 1, 3)
    )  # [B, J, T, F]
    return [
        Xt[c * BS : (c + 1) * BS].reshape(P, T, F) for c in range(CORES)
    ]


def _unshard(parts: list[np.ndarray]) -> np.ndarray:
    """per-core [(b j), T, F] -> [B, T, N]."""
    full = np.stack(parts).reshape(B, J, T, F)
    return np.ascontiguousarray(full.transpose(0, 2, 1, 3)).reshape(B, T, N)


def _run(X: np.ndarray, w: np.ndarray, **spmd_kwargs):
    X = np.asarray(X, dtype=np.float32)
    inv_tau = float(1.0 / (1.0 + np.exp(-np.float64(np.asarray(w).item()))))
    nc = _build(inv_tau)
    in_maps = [{"x": xs} for xs in _shard(X)]
    res = run_bass_kernel_spmd(nc, in_maps, core_ids=list(range(CORES)), **spmd_kwargs)
    spikes = _unshard(
        [np.asarray(res.results[c]["spk"]).astype(np.float32) for c in range(CORES)]
    )
    i_pot = _unshard([res.results[c]["oi"] for c in range(CORES)])
    return (spikes, i_pot), res


def kernel(X: np.ndarray, w: np.ndarray):
    out, _ = _run(X, w)
    return out


# revision 66
# speedup vs baseline: 1.0135x; 1.0135x over previous
"""AxonLIFNode forward on 8 Trainium2 NeuronCores.

Reference recurrence (per element, sequential over T):
    mem   = mem + (x_t + V_RESET - mem) / TAU        # V_RESET=0, TAU=2
    spike = (mem - V_TH > 0)                         # V_TH=1, {0.0, 1.0}
    mem   = (1 - spike) * mem + V_RESET * spike      # reset to 0 on spike
    out_i = out_i * sigmoid(w) + spike               # axon current (w=0 -> 0.5)
    outputs: (spike, out_i), both [B, T, N] f32

Strategy: data-parallel over the batch axis (B=64 -> 8 per core). Per core the
32768 independent series are laid out as 128 partitions x 256 free elements;
the T=64 loop runs as a serial chain of one fused custom-DVE instruction per
timestep computing the pre-reset membrane m1_t from (x_t, m1_{t-1}):

    prev = m1 * (m1 <= 1)            # reset of the *previous* pre-reset mem
    m1'  = prev + (x - prev) * 0.5

which is bit-exact vs. the reference ordering (each ALU stage is one IEEE f32
rounding; *0.5 == /2 exactly). A second fused DVE op produces the axon current
oi_t = oi_{t-1}*inv_tau + (m1_t > 1) directly from m1, so the DVE critical
path is exactly two 1x-mode instructions per timestep (the ISA floor). Spikes
are produced off the critical path on the Scalar(ACT) engine with a saturated
sigmoid (exact {0,1}, see _build) and stored as fp8. X streams in on the SP
HWDGE ring in tiered batches; spikes stream out per-G-timestep group on the
ACT ring while the heavier oi stream is staged across two groups (1 MB
stores, late ones on the SP ring once inputs finish).
"""

import numpy as np

import concourse.bacc as bacc
import concourse.mybir as mybir
import concourse.dve_ops as dve_ops
from concourse.dve_ops import DveOp
from concourse.dve_spec import Spec, Src0, Src1, C0, C1, lower
from concourse.dve_uop import DveOpSpec
from concourse.tile import TileContext
from concourse.bass_utils import run_bass_kernel_spmd

# Problem shape (hardcoded per harness contract).
B, T, N = 64, 64, 4096
CORES = 8
BS = B // CORES          # batches per core
P = 128                  # SBUF partitions
J = 16                   # n-chunks per batch: BS * J == P
F = N // J               # free elements per partition per timestep (256)
G = 4                    # timesteps per output staging group
GIN = 16                 # timesteps per input DMA batch
SPK_SCALE = 1.0e9        # sigmoid saturation trick scale (see _build)

def _register_op(name: str, spec: Spec) -> DveOp:
    """Register a custom DVE op in the global registry with a computed sha."""
    for op in dve_ops.OPS:
        if op.name == name:
            return op
    row = dve_ops._CUSTOM_DVE_ROW_BASE + len(dve_ops.OPS)
    assert row < 0x20, "custom-DVE opcode rows exhausted"
    shas = {}
    for ver in ("v3", "v4"):
        uops = lower(spec, ver=ver)
        shas[ver] = DveOpSpec(name=name, opcode=row, uops=uops, rd1_en=True).sha(ver)
    op = DveOp(name, spec, subdim=False, uops_sha=shas)
    dve_ops._SUB_OPCODE_FOR_NAME[name] = row
    dve_ops.OPS.append(op)
    dve_ops.CUSTOM_DVE_SPECS[name] = spec
    return op


def _lif_ops() -> tuple[DveOp, DveOp]:
    """LIF_M1: m1_t from (x_t, m1_{t-1}); LIF_OI: oi_t from (oi_{t-1}, m1_t).

    LIF_M1: out = prev + (Src0 - prev) * C0, prev = Src1 * (Src1 <= C1)
    LIF_OI: out = Src0 * C0 + (Src1 > C1)
    Each ALU stage is one IEEE f32 rounding; bit-exact vs the reference.
    """
    keep = Src1 <= C1
    prev = Src1 * keep
    m1 = _register_op(
        "LIF_M1_ANT",
        Spec(
            body=prev + (Src0 - prev) * C0,
            reference=lambda in0, in1, s0, s1, imm2: (
                (p := (in1 * (in1 <= s1)).astype(np.float32))
                + (in0 - p) * np.float32(s0)
            ).astype(np.float32),
        ),
    )
    oi = _register_op(
        "LIF_OI_ANT",
        Spec(
            body=Src0 * C0 + (Src1 > C1),
            reference=lambda in0, in1, s0, s1, imm2: (
                in0 * np.float32(s0) + (in1 > s1)
            ).astype(np.float32),
        ),
    )
    return m1, oi


_nc_cache: dict = {}


def _build(inv_tau: float):
    """Trace + compile the per-core Bass program (SPMD: same NEFF, 8 cores)."""
    key = float(inv_tau)
    if key in _nc_cache:
        return _nc_cache[key]

    lif_m1, lif_oi = _lif_ops()
    f32 = mybir.dt.float32
    fp8 = mybir.dt.float8e4

    nc = bacc.Bacc(
        "TRN2",
        target_bir_lowering=False,
        debug=False,
        enable_asserts=False,
        num_devices=CORES,
    )
    # Host pre-transposes each core's shard to [(b j) = 128, T, F] contiguous,
    # so every DMA is a 3-dim AP with an 8 KB contiguous run per partition.
    x_r = nc.dram_tensor("x", [P, T, F], f32, kind="ExternalInput").ap()
    # Spikes are exactly {0.0, 1.0}: store as fp8-e4m3 (lossless) to cut the
    # HBM write traffic 4x; the host upcasts to f32.
    spk_r = nc.dram_tensor("spk", [P, T, F], fp8, kind="ExternalOutput").ap()
    oi_r = nc.dram_tensor("oi", [P, T, F], f32, kind="ExternalOutput").ap()

    with TileContext(nc) as tc:
        with (
            tc.tile_pool(name="const", bufs=1) as cpool,
            tc.tile_pool(name="xin", bufs=1) as xpool,
            tc.tile_pool(name="sout", bufs=6) as spool,
            tc.tile_pool(name="oout", bufs=1) as opool,
            tc.tile_pool(name="mstate", bufs=10) as mpool,
        ):
            m_init = cpool.tile([P, F], f32)
            nc.vector.memset(m_init[:], 0.0)
            # Spike via one ACT op: sigmoid(S*m1 - (S + 64)) with S = 1e9.
            # fl(S*m1) quantizes to a 64-ulp grid around S, so the argument is
            # always <= -64 (no spike, incl. m1 == V_TH exactly -> -64) or
            # >= +64 (spike); sigmoid saturates to 0.0 / 1.0 there.
            spk_bias = cpool.tile([P, 1], f32)
            nc.vector.memset(spk_bias[:], -(SPK_SCALE + 64.0))
            m_prev = m_init[:]

            # The whole per-core X fits in SBUF (64 KB/partition). Stage it in
            # tiered batches — small first chunks so compute starts early —
            # and issue every input DMA up front on the dedicated SP ring.
            in_batches = [1, 3, 4, 8] + [GIN] * ((T - 16) // GIN)
            assert sum(in_batches) == T
            x_tiles = []  # (tile, t_start, t_len)
            t_cursor = 0
            for blen in in_batches:
                xt = xpool.tile([P, blen, F], f32, name=f"x_{t_cursor}", bufs=1)
                x_tiles.append((xt, t_cursor, blen))
                t_cursor += blen

            issued = set()

            def issue_x_until(t_max):
                # Pace input DMAs so they don't flood the SDMA engines and
                # starve the output stream. The first small batches alternate
                # between the two HWDGE rings (the ACT ring is idle until the
                # first outputs), so their transfers run in parallel and the
                # DVE chain never waits at an early batch boundary.
                for bi, (xt, ts, tl) in enumerate(x_tiles):
                    if ts < t_max and bi not in issued:
                        issued.add(bi)
                        nc.sync.dma_start(out=xt[:], in_=x_r[:, ts : ts + tl, :])

            def x_slice(t):
                for xt, ts, tl in x_tiles:
                    if ts <= t < ts + tl:
                        return xt[:, t - ts, :]
                raise AssertionError(t)

            issue_x_until(16)
            # One linear oi buffer [P, T+1, F]; slot 0 is the zero initial
            # state, oi_t lives at slot t+1. A single LIF_OI op then covers
            # TWO timesteps: out = slots [t+1, t+2], in0 = slots [t, t+1] —
            # the second half of the output stream reads the oi_t values the
            # same instruction wrote ~F cycles earlier (far beyond the DVE's
            # pipeline depth), halving the per-instruction fixed cost.
            oi_buf = opool.tile([P, T + 1, F], f32)
            nc.vector.memset(oi_buf[:, 0, :], 0.0)
            for g in range(T // G):
                issue_x_until((g + 3) * G)
                s_t = spool.tile([P, G, F], fp8)
                m_t = mpool.tile([P, G, F], f32)
                for k in range(G):
                    # DVE: m1_t = prev + (x_t - prev)/TAU, prev = reset(m1_{t-1})
                    nc.vector._custom_dve(
                        lif_m1,
                        out=m_t[:, k, :],
                        in0=x_slice(g * G + k),
                        in1=m_prev,
                        s0=0.5,      # 1/TAU
                        s1=1.0,      # V_TH
                    )
                    m_prev = m_t[:, k, :]
                    if k % 2 == 1:
                        # DVE: oi for timesteps (t-1, t) in one op
                        t0 = g * G + k - 1
                        nc.vector._custom_dve(
                            lif_oi,
                            out=oi_buf[:, t0 + 1 : t0 + 3, :],
                            in0=oi_buf[:, t0 : t0 + 2, :],
                            in1=m_t[:, k - 1 : k + 1, :],
                            s0=inv_tau,
                            s1=1.0,
                        )
                # ACT: spike = sigmoid(S*m1 - (S+64)) in {0, 1}. One batched op
                # per group; the final group goes half-at-a-time so the last
                # spike store isn't serialized behind a full-group ACT op.
                if g == T // G - 1:
                    h = G // 2
                    nc.scalar.activation(
                        out=s_t[:, :h, :],
                        in_=m_t[:, :h, :],
                        func=mybir.ActivationFunctionType.Sigmoid,
                        bias=spk_bias[:],
                        scale=SPK_SCALE,
                    )
                    nc.scalar.activation(
                        out=s_t[:, h:, :],
                        in_=m_t[:, h:, :],
                        func=mybir.ActivationFunctionType.Sigmoid,
                        bias=spk_bias[:],
                        scale=SPK_SCALE,
                    )
                else:
                    nc.scalar.activation(
                        out=s_t[:],
                        in_=m_t[:],
                        func=mybir.ActivationFunctionType.Sigmoid,
                        bias=spk_bias[:],
                        scale=SPK_SCALE,
                    )
                # Early groups: outputs on the ACT ring while inputs own the SP
                # ring; once inputs are done the SP ring takes the oi stream.
                if g == T // G - 1:
                    h = G // 2
                    nc.scalar.dma_start(
                        out=spk_r[:, g * G : g * G + h, :], in_=s_t[:, :h, :]
                    )
                    nc.scalar.dma_start(
                        out=spk_r[:, g * G + h : (g + 1) * G, :], in_=s_t[:, h:, :]
                    )
                else:
                    nc.scalar.dma_start(
                        out=spk_r[:, g * G : (g + 1) * G, :], in_=s_t[:]
                    )
                if g == T // G - 1:
                    # Final stores split across both rings so the tail drains
                    # in parallel.
                    nc.sync.dma_start(
                        out=oi_r[:, g * G - G : g * G, :],
                        in_=oi_buf[:, g * G - G + 1 : g * G + 1, :],
                    )
                    nc.scalar.dma_start(
                        out=oi_r[:, g * G : (g + 1) * G, :],
                        in_=oi_buf[:, g * G + 1 : (g + 1) * G + 1, :],
                    )
                elif g % 2 == 1:
                    oi_eng = nc.sync if g >= (T // G) // 2 else nc.scalar
                    oi_eng.dma_start(
                        out=oi_r[:, (g - 1) * G : (g + 1) * G, :],
                        in_=oi_buf[:, (g - 1) * G + 1 : (g + 1) * G + 1, :],
                    )

    nc.compile()
    _nc_cache[key] = nc
    return nc


def _shard(X: np.ndarray) -> list[np.ndarray]:
    """[B, T, N] -> per-core [(b j) = 128, T, F] contiguous."""
    Xt = np.ascontiguousarray(
        X.reshape(B, T, J, F).transpose(0, 2, 1, 3)
    )  # [B, J, T, F]
    return [
        Xt[c * BS : (c + 1) * BS].reshape(P, T, F) for c in range(CORES)
    ]


def _unshard(parts: list[np.ndarray]) -> np.ndarray:
    """per-core [(b j), T, F] -> [B, T, N]."""
    full = np.stack(parts).reshape(B, J, T, F)
    return np.ascontiguousarray(full.transpose(0, 2, 1, 3)).reshape(B, T, N)


def _run(X: np.ndarray, w: np.ndarray, **spmd_kwargs):
    X = np.asarray(X, dtype=np.float32)
    inv_tau = float(1.0 / (1.0 + np.exp(-np.float64(np.asarray(w).item()))))
    nc = _build(inv_tau)
    in_maps = [{"x": xs} for xs in _shard(X)]
    res = run_bass_kernel_spmd(nc, in_maps, core_ids=list(range(CORES)), **spmd_kwargs)
    spikes = _unshard(
        [np.asarray(res.results[c]["spk"]).astype(np.float32) for c in range(CORES)]
    )
    i_pot = _unshard([res.results[c]["oi"] for c in range(CORES)])
    return (spikes, i_pot), res


def kernel(X: np.ndarray, w: np.ndarray):
    out, _ = _run(X, w)
    return out


# revision 67
# speedup vs baseline: 1.0622x; 1.0480x over previous
"""AxonLIFNode forward on 8 Trainium2 NeuronCores.

Reference recurrence (per element, sequential over T):
    mem   = mem + (x_t + V_RESET - mem) / TAU        # V_RESET=0, TAU=2
    spike = (mem - V_TH > 0)                         # V_TH=1, {0.0, 1.0}
    mem   = (1 - spike) * mem + V_RESET * spike      # reset to 0 on spike
    out_i = out_i * sigmoid(w) + spike               # axon current (w=0 -> 0.5)
    outputs: (spike, out_i), both [B, T, N] f32

Strategy: data-parallel over the batch axis (B=64 -> 8 per core). Per core the
32768 independent series are laid out as 128 partitions x 256 free elements;
the T=64 loop runs as a serial chain of one fused custom-DVE instruction per
timestep computing the pre-reset membrane m1_t from (x_t, m1_{t-1}):

    prev = m1 * (m1 <= 1)            # reset of the *previous* pre-reset mem
    m1'  = prev + (x - prev) * 0.5

which is bit-exact vs. the reference ordering (each ALU stage is one IEEE f32
rounding; *0.5 == /2 exactly). A second fused DVE op produces the axon current
oi_t = oi_{t-1}*inv_tau + (m1_t > 1) directly from m1, so the DVE critical
path is exactly two 1x-mode instructions per timestep (the ISA floor). Spikes
are produced off the critical path on the Scalar(ACT) engine with a saturated
sigmoid (exact {0,1}, see _build) and stored as fp8. X streams in on the SP
HWDGE ring in tiered batches; spikes stream out per-G-timestep group on the
ACT ring while the heavier oi stream is staged across two groups (1 MB
stores, late ones on the SP ring once inputs finish).
"""

import numpy as np

import concourse.bacc as bacc
import concourse.mybir as mybir
import concourse.dve_ops as dve_ops
from concourse.dve_ops import DveOp
from concourse.dve_spec import Spec, Src0, Src1, C0, C1, lower
from concourse.dve_uop import DveOpSpec
from concourse.tile import TileContext
from concourse.bass_utils import run_bass_kernel_spmd

# Problem shape (hardcoded per harness contract).
B, T, N = 64, 64, 4096
CORES = 8
BS = B // CORES          # batches per core
P = 128                  # SBUF partitions
J = 16                   # n-chunks per batch: BS * J == P
F = N // J               # free elements per partition per timestep (256)
G = 4                    # timesteps per output staging group
GIN = 16                 # timesteps per input DMA batch
SPK_SCALE = 1.0e9        # sigmoid saturation trick scale (see _build)

def _register_op(name: str, spec: Spec) -> DveOp:
    """Register a custom DVE op in the global registry with a computed sha."""
    for op in dve_ops.OPS:
        if op.name == name:
            return op
    row = dve_ops._CUSTOM_DVE_ROW_BASE + len(dve_ops.OPS)
    assert row < 0x20, "custom-DVE opcode rows exhausted"
    shas = {}
    for ver in ("v3", "v4"):
        uops = lower(spec, ver=ver)
        shas[ver] = DveOpSpec(name=name, opcode=row, uops=uops, rd1_en=True).sha(ver)
    op = DveOp(name, spec, subdim=False, uops_sha=shas)
    dve_ops._SUB_OPCODE_FOR_NAME[name] = row
    dve_ops.OPS.append(op)
    dve_ops.CUSTOM_DVE_SPECS[name] = spec
    return op


def _lif_ops() -> tuple[DveOp, DveOp]:
    """LIF_M1: m1_t from (x_t, m1_{t-1}); LIF_OI: oi_t from (oi_{t-1}, m1_t).

    LIF_M1: out = prev + (Src0 - prev) * C0, prev = Src1 * (Src1 <= C1)
    LIF_OI: out = Src0 * C0 + (Src1 > C1)
    Each ALU stage is one IEEE f32 rounding; bit-exact vs the reference.
    """
    keep = Src1 <= C1
    prev = Src1 * keep
    m1 = _register_op(
        "LIF_M1_ANT",
        Spec(
            body=prev + (Src0 - prev) * C0,
            reference=lambda in0, in1, s0, s1, imm2: (
                (p := (in1 * (in1 <= s1)).astype(np.float32))
                + (in0 - p) * np.float32(s0)
            ).astype(np.float32),
        ),
    )
    oi = _register_op(
        "LIF_OI_ANT",
        Spec(
            body=Src0 * C0 + (Src1 > C1),
            reference=lambda in0, in1, s0, s1, imm2: (
                in0 * np.float32(s0) + (in1 > s1)
            ).astype(np.float32),
        ),
    )
    return m1, oi


_nc_cache: dict = {}


def _build(inv_tau: float):
    """Trace + compile the per-core Bass program (SPMD: same NEFF, 8 cores)."""
    key = float(inv_tau)
    if key in _nc_cache:
        return _nc_cache[key]

    lif_m1, lif_oi = _lif_ops()
    f32 = mybir.dt.float32
    fp8 = mybir.dt.float8e4

    nc = bacc.Bacc(
        "TRN2",
        target_bir_lowering=False,
        debug=False,
        enable_asserts=False,
        num_devices=CORES,
    )
    # Host pre-transposes each core's shard to [(b j) = 128, T, F] contiguous,
    # so every DMA is a 3-dim AP with an 8 KB contiguous run per partition.
    x_r = nc.dram_tensor("x", [P, T, F], f32, kind="ExternalInput").ap()
    # Spikes are exactly {0.0, 1.0}: store as fp8-e4m3 (lossless) to cut the
    # HBM write traffic 4x; the host upcasts to f32.
    spk_r = nc.dram_tensor("spk", [P, T, F], fp8, kind="ExternalOutput").ap()
    oi_r = nc.dram_tensor("oi", [P, T, F], f32, kind="ExternalOutput").ap()

    with TileContext(nc) as tc:
        with (
            tc.tile_pool(name="const", bufs=1) as cpool,
            tc.tile_pool(name="xin", bufs=1) as xpool,
            tc.tile_pool(name="sout", bufs=6) as spool,
            tc.tile_pool(name="oout", bufs=1) as opool,
            tc.tile_pool(name="mstate", bufs=10) as mpool,
        ):
            m_init = cpool.tile([P, F], f32)
            nc.vector.memset(m_init[:], 0.0)
            # Spike via one ACT op: sigmoid(S*m1 - (S + 64)) with S = 1e9.
            # fl(S*m1) quantizes to a 64-ulp grid around S, so the argument is
            # always <= -64 (no spike, incl. m1 == V_TH exactly -> -64) or
            # >= +64 (spike); sigmoid saturates to 0.0 / 1.0 there.
            spk_bias = cpool.tile([P, 1], f32)
            nc.vector.memset(spk_bias[:], -(SPK_SCALE + 64.0))
            m_prev = m_init[:]

            # The whole per-core X fits in SBUF (64 KB/partition). Stage it in
            # tiered batches — small first chunks so compute starts early —
            # and issue every input DMA up front on the dedicated SP ring.
            in_batches = [1, 3, 4, 8] + [GIN] * ((T - 16) // GIN)
            assert sum(in_batches) == T
            x_tiles = []  # (tile, t_start, t_len)
            t_cursor = 0
            for blen in in_batches:
                xt = xpool.tile([P, blen, F], f32, name=f"x_{t_cursor}", bufs=1)
                x_tiles.append((xt, t_cursor, blen))
                t_cursor += blen

            issued = set()

            def issue_x_until(t_max):
                # Pace input DMAs so they don't flood the SDMA engines and
                # starve the output stream. The first small batches alternate
                # between the two HWDGE rings (the ACT ring is idle until the
                # first outputs), so their transfers run in parallel and the
                # DVE chain never waits at an early batch boundary.
                for bi, (xt, ts, tl) in enumerate(x_tiles):
                    if ts < t_max and bi not in issued:
                        issued.add(bi)
                        nc.sync.dma_start(out=xt[:], in_=x_r[:, ts : ts + tl, :])

            def x_slice(t):
                for xt, ts, tl in x_tiles:
                    if ts <= t < ts + tl:
                        return xt[:, t - ts, :]
                raise AssertionError(t)

            issue_x_until(16)
            # One linear oi buffer [P, T+1, F]; slot 0 is the zero initial
            # state, oi_t lives at slot t+1. A single LIF_OI op then covers
            # TWO timesteps: out = slots [t+1, t+2], in0 = slots [t, t+1] —
            # the second half of the output stream reads the oi_t values the
            # same instruction wrote ~F cycles earlier (far beyond the DVE's
            # pipeline depth), halving the per-instruction fixed cost.
            oi_buf = opool.tile([P, T + 1, F], f32)
            nc.vector.memset(oi_buf[:, 0, :], 0.0)
            for g in range(T // G):
                issue_x_until((g + 3) * G)
                s_t = spool.tile([P, G, F], fp8)
                m_t = mpool.tile([P, G, F], f32)
                for k in range(G):
                    # DVE: m1_t = prev + (x_t - prev)/TAU, prev = reset(m1_{t-1})
                    nc.vector._custom_dve(
                        lif_m1,
                        out=m_t[:, k, :],
                        in0=x_slice(g * G + k),
                        in1=m_prev,
                        s0=0.5,      # 1/TAU
                        s1=1.0,      # V_TH
                    )
                    m_prev = m_t[:, k, :]
                # DVE: all G oi timesteps of the group in ONE op — the output
                # stream's later quarters read the oi values written F elements
                # earlier in the same instruction (SBUF RAW, ~256-cycle margin)
                t0 = g * G
                nc.vector._custom_dve(
                    lif_oi,
                    out=oi_buf[:, t0 + 1 : t0 + 1 + G, :],
                    in0=oi_buf[:, t0 : t0 + G, :],
                    in1=m_t[:],
                    s0=inv_tau,
                    s1=1.0,
                )
                # ACT: spike = sigmoid(S*m1 - (S+64)) in {0, 1}. One batched op
                # per group; the final group goes half-at-a-time so the last
                # spike store isn't serialized behind a full-group ACT op.
                if g == T // G - 1:
                    h = G // 2
                    nc.scalar.activation(
                        out=s_t[:, :h, :],
                        in_=m_t[:, :h, :],
                        func=mybir.ActivationFunctionType.Sigmoid,
                        bias=spk_bias[:],
                        scale=SPK_SCALE,
                    )
                    nc.scalar.activation(
                        out=s_t[:, h:, :],
                        in_=m_t[:, h:, :],
                        func=mybir.ActivationFunctionType.Sigmoid,
                        bias=spk_bias[:],
                        scale=SPK_SCALE,
                    )
                else:
                    nc.scalar.activation(
                        out=s_t[:],
                        in_=m_t[:],
                        func=mybir.ActivationFunctionType.Sigmoid,
                        bias=spk_bias[:],
                        scale=SPK_SCALE,
                    )
                # Early groups: outputs on the ACT ring while inputs own the SP
                # ring; once inputs are done the SP ring takes the oi stream.
                if g == T // G - 1:
                    h = G // 2
                    nc.scalar.dma_start(
                        out=spk_r[:, g * G : g * G + h, :], in_=s_t[:, :h, :]
                    )
                    nc.scalar.dma_start(
                        out=spk_r[:, g * G + h : (g + 1) * G, :], in_=s_t[:, h:, :]
                    )
                else:
                    nc.scalar.dma_start(
                        out=spk_r[:, g * G : (g + 1) * G, :], in_=s_t[:]
                    )
                if g == T // G - 1:
                    # Final stores split across both rings so the tail drains
                    # in parallel.
                    nc.sync.dma_start(
                        out=oi_r[:, g * G - G : g * G, :],
                        in_=oi_buf[:, g * G - G + 1 : g * G + 1, :],
                    )
                    nc.scalar.dma_start(
                        out=oi_r[:, g * G : (g + 1) * G, :],
                        in_=oi_buf[:, g * G + 1 : (g + 1) * G + 1, :],
                    )
                elif g % 2 == 1:
                    oi_eng = nc.sync if g >= (T // G) // 2 else nc.scalar
                    oi_eng.dma_start(
                        out=oi_r[:, (g - 1) * G : (g + 1) * G, :],
                        in_=oi_buf[:, (g - 1) * G + 1 : (g + 1) * G + 1, :],
                    )

    nc.compile()
    _nc_cache[key] = nc
    return nc


def _shard(X: np.ndarray) -> list[np.ndarray]:
    """[B, T, N] -> per-core [(b j) = 128, T, F] contiguous."""
    Xt = np.ascontiguousarray(
        X.reshape(B, T, J, F).transpose(0, 2, 1, 3)
    )  # [B, J, T, F]
    return [
        Xt[c * BS : (c + 1) * BS].reshape(P, T, F) for c in range(CORES)
    ]


def _unshard(parts: list[np.ndarray]) -> np.ndarray:
    """per-core [(b j), T, F] -> [B, T, N]."""
    full = np.stack(parts).reshape(B, J, T, F)
    return np.ascontiguousarray(full.transpose(0, 2, 1, 3)).reshape(B, T, N)


def _run(X: np.ndarray, w: np.ndarray, **spmd_kwargs):
    X = np.asarray(X, dtype=np.float32)
    inv_tau = float(1.0 / (1.0 + np.exp(-np.float64(np.asarray(w).item()))))
    nc = _build(inv_tau)
    in_maps = [{"x": xs} for xs in _shard(X)]
    res = run_bass_kernel_spmd(nc, in_maps, core_ids=list(range(CORES)), **spmd_kwargs)
    spikes = _unshard(
        [np.asarray(res.results[c]["spk"]).astype(np.float32) for c in range(CORES)]
    )
    i_pot = _unshard([res.results[c]["oi"] for c in range(CORES)])
    return (spikes, i_pot), res


def kernel(X: np.ndarray, w: np.ndarray):
    out, _ = _run(X, w)
    return out
